# revision 2
# baseline (speedup 1.0000x reference)
"""LSTM decoder (nn_Decoder) on 8 trn2 NeuronCores — dual-pipeline version.

Tensor-parallel over the gate dimension (each core owns a 128-row slice of
h/c) with the batch split into two independent half-recurrences (rows 0:32
and 32:64) run phase-shifted, so each half's h-broadcast flight hides under
the other half's compute.

Transposed layout throughout: partitions = this core's 128 h dims, free dim
= batch half. Gates are computed as gates.T chunks f,i,g,o, each [128, 32],
via 8 k-matmuls with the weight tile [128,128] bf16 as lhsT (stationary) and
the gathered hT slot [128, 32] as rhs — measured 25 ns/kmm. cT/hT stay
transposed, so there are no PE transposes, and gate biases ride the ACT
per-partition bias port (no bias matmuls).

PSUM banks: per half, bankX = f(cols 0:32)+g(32:64), bankY = i(0:32)+o(32:64)
so the PE never writes a bank an ACT read is pending on (g waits sig_f, o
waits sig_i). outproj double-buffers 2 more banks ([64,64], lhsT = gather
slot [128,64] spanning both halves, batch rows come out in order).

Exchange: per half per step one remote_dma_broadcast of hT [128,32] bf16
(8 KB) into slot-column pid*64 + 32*half of every core's gather ring (depth
4), on SWDGE queue <half>. Measured ring: trigger->all-arrived ~2.6 us.

Semaphores per half h: gs[h][parity] arrivals (+16/round), ls[h] local send
(+16), pp[h] prep, gk[h] gate chunks (h0=1; f,i,g,o of step t = 4t-2..4t+1),
ak[h] ACT ops (f,i,g,o,c of step t = 5t-4..5t), dk[h] DVE ops (t=1: c=1,
hT=2; t>=2: c2=4t-5, t1=4t-4, c=4t-3, hT=4t-2), hs[h] h0 ready.
Shared: osem/ocp outproj produce/copy (= j), in_dma, odma.
"""
import os
import sys

sys.path.insert(0, "/opt/trn_rl_repo")

import numpy as np
import ml_dtypes

BF16 = ml_dtypes.bfloat16

B = 64          # batch
BH = 32         # batch half
L = 256         # latent dim
H = 1024        # hidden
O = 512         # output dim
S = 256         # seq len
NC = 8          # cores
HL = H // NC    # 128, per-core h slice
OL = O // NC    # 64, per-core out slice

# gate chunk order (compute order): f first (c update), then i, g, o.
# chunk c -> quarter of the 4H gate dim in reference i,f,g,o order.
CHUNK_Q = [1, 0, 2, 3]   # f, i, g, o


def GK(t, c):
    return 4 * t - 2 + c          # chunk c in 0..3; h0 = 1


def AK(t, c):
    return 5 * t - 5 + c + 1      # c in 0..4 = f,i,g,o,tanh_c


def DVE_C(t):
    return 1 if t == 1 else 4 * t - 3


def DVE_H(t):
    return 2 if t == 1 else 4 * t - 2


def _build_nc(s_len):
    from concourse import bass, mybir
    from concourse import bacc
    from concourse.ap import AP
    from contextlib import ExitStack

    S_ = s_len
    nc = bacc.Bacc("TRN2", debug=False, num_swdge_queues=2)
    f32 = mybir.dt.float32
    bf16 = mybir.dt.bfloat16
    AF = mybir.ActivationFunctionType
    ALU = mybir.AluOpType

    d_lat = nc.dram_tensor("latT", [128, 2 * B], bf16, kind="ExternalInput")
    d_fcw = nc.dram_tensor("fcwT", [128, 2 * HL], bf16, kind="ExternalInput")
    d_fcb = nc.dram_tensor("fcb", [128, 1], f32, kind="ExternalInput")
    d_wc = nc.dram_tensor("wcT", [128, 32 * 128], bf16, kind="ExternalInput")
    d_whh = nc.dram_tensor("whhT", [128, 32 * 128], bf16, kind="ExternalInput")
    d_outw = nc.dram_tensor("outwT", [128, NC * OL], bf16, kind="ExternalInput")
    d_bias = nc.dram_tensor("biasT", [128, 4], f32, kind="ExternalInput")
    d_misc = nc.dram_tensor("misc", [1, 2 * B], bf16, kind="ExternalInput")
    d_out = nc.dram_tensor("out", [B, S_ * OL], f32, kind="ExternalOutput")

    ctx = ExitStack()
    sem = lambda n: ctx.enter_context(nc.semaphore(n))
    sb = lambda n, sh, dt: ctx.enter_context(nc.sbuf_tensor(n, sh, dt))
    ps = lambda n, sh, dt: ctx.enter_context(nc.psum_tensor(n, sh, dt))

    in_dma = sem("in_dma")     # 8 loads x 16 = 128
    gs = [[sem(f"gs{h}p{p}") for p in range(2)] for h in range(2)]
    ls = [sem(f"ls{h}") for h in range(2)]
    pp = [sem(f"pp{h}") for h in range(2)]
    gk = [sem(f"gk{h}") for h in range(2)]
    ak = [sem(f"ak{h}") for h in range(2)]
    dk = [sem(f"dk{h}") for h in range(2)]
    hs = [sem(f"hs{h}") for h in range(2)]
    osem = sem("osem")
    ocp = sem("ocp")
    odma = sem("odma")

    lat_sb = sb("lat_sb", [128, 2 * B], bf16)
    fcw_sb = sb("fcw_sb", [128, 2 * HL], bf16)
    fcb_sb = sb("fcb_sb", [128, 1], f32)
    wc_sb = sb("wc_sb", [128, 32 * 128], bf16)
    whh_sb = sb("whh_sb", [128, 32 * 128], bf16)
    outw_sb = sb("outw_sb", [128, NC * OL], bf16)
    bias_sb = sb("bias_sb", [128, 4], f32)
    misc_sb = sb("misc_sb", [1, 2 * B], bf16)
    gather = [sb(f"gather{r}", [128, NC * B], bf16) for r in range(4)]
    hT = [[sb(f"hT{h}p{p}", [128, BH], bf16) for p in range(2)]
          for h in range(2)]
    sfT = [sb(f"sfT{h}", [128, BH], f32) for h in range(2)]
    siT = [sb(f"siT{h}", [128, BH], f32) for h in range(2)]
    sgT = [sb(f"sgT{h}", [128, BH], f32) for h in range(2)]
    soT = [sb(f"soT{h}", [128, BH], f32) for h in range(2)]
    tanhc = [sb(f"tanhc{h}", [128, BH], f32) for h in range(2)]
    c2 = [sb(f"c2_{h}", [128, BH], f32) for h in range(2)]
    t1 = [sb(f"t1_{h}", [128, BH], f32) for h in range(2)]
    cT = [sb(f"cT{h}", [128, BH], f32) for h in range(2)]
    out_acc = sb("out_acc", [B, S_ * OL], f32)

    px = [ps(f"px{h}", [128, 2 * BH], f32) for h in range(2)]   # f | g
    py = [ps(f"py{h}", [128, 2 * BH], f32) for h in range(2)]   # i | o
    pout = [ps(f"pout{p}", [B, OL], f32) for p in range(2)]

    ones = lambda: misc_sb[0:1, 0:B]
    outb = lambda: misc_sb[0:1, B:B + OL]

    OUT_CHUNK = min(32, S_)
    n_chunks = (S_ + OUT_CHUNK - 1) // OUT_CHUNK

    with nc.Block() as block:

        @block.sync
        def _(sync):
            n = 0
            for dram, buf in ((d_lat, lat_sb), (d_fcw, fcw_sb),
                              (d_fcb, fcb_sb), (d_wc, wc_sb),
                              (d_whh, whh_sb), (d_outw, outw_sb),
                              (d_bias, bias_sb), (d_misc, misc_sb)):
                if n:
                    sync.wait_ge(in_dma, n)
                sync.dma_start(buf[:, :], dram[:, :]).then_inc(in_dma, 16)
                n += 16
            for ch in range(n_chunks):
                hi = min((ch + 1) * OUT_CHUNK, S_)   # steps 1..hi copied
                sync.wait_ge(ocp, hi)
                if ch:
                    sync.wait_ge(odma, 16 * ch)
                sync.dma_start(
                    d_out[:, ch * OUT_CHUNK * OL:hi * OL],
                    out_acc[:, ch * OUT_CHUNK * OL:hi * OL],
                ).then_inc(odma, 16)
            sync.wait_ge(odma, 16 * n_chunks)

        @block.gpsimd
        def _(gp):
            pid = gp.partition_id()
            dyn = [[AP(gather[r].ap().tensor, pid * B + BH * h,
                       gather[r][:, 0:BH].ap.copy()) for r in range(4)]
                   for h in range(2)]
            gp.wait_ge(in_dma, 128)
            gp.bir_kernel_barrier_wait([list(range(NC))])
            for i in range(S_ + 1):
                for h in range(2):
                    if i:
                        gp.wait_ge(ls[h], 16 * i)
                    gp.remote_dma_broadcast(
                        dyn[h][i % 4], hT[h][i % 2][:, :],
                        remote_sem=gs[h][i % 2], local_sem=ls[h],
                        rdests=[(0, k) for k in range(NC)],
                        queue_num=h,
                    ).then_inc(pp[h], 1)
                    gp.wait_ge(pp[h], i + 1)
                    if i == 0:
                        gp.wait_ge(hs[h], 1)
                    else:
                        gp.wait_ge(dk[h], DVE_H(i))
                    gp.trigger_dma(count=1, queue_num=h)

        @block.tensor
        def _(te):
            mm = te.matmul

            def outproj(j):
                po = pout[j % 2]
                # needs BOTH halves' exchange j arrived; half A's wait is
                # implied by program order, half B's is explicit (cheap —
                # B's round j lands well before A's round j+1 in steady
                # state, so this almost never stalls).
                te.wait_ge(gs[0][j % 2], 16 * (j // 2 + 1))
                te.wait_ge(gs[1][j % 2], 16 * (j // 2 + 1))
                if j > 2:
                    te.wait_ge(ocp, j - 2)
                mm(po[:, :], ones(), outb(), start=True, stop=False)
                gb = gather[j % 4]
                for k in range(NC):
                    last = k == NC - 1
                    ins = mm(po[:, :], gb[:, k * B:(k + 1) * B],
                             outw_sb[:, k * OL:(k + 1) * OL],
                             start=False, stop=last)
                    if last:
                        ins.then_inc(osem)           # osem = j

            te.wait_ge(in_dma, 128)
            for h in range(2):
                # h0T slice = fc_w @ latent.T + (fcb on ACT)
                mm(px[h][:, 0:BH], fcw_sb[:, 0:HL],
                   lat_sb[:, BH * h:BH * h + BH], start=True, stop=False)
                mm(px[h][:, 0:BH], fcw_sb[:, HL:2 * HL],
                   lat_sb[:, B + BH * h:B + BH * h + BH],
                   start=False, stop=True).then_inc(gk[h])      # gk = 1

            for t in range(1, S_ + 1):
                for h in range(2):
                    W = whh_sb if t == 1 else wc_sb
                    if t == 1:
                        te.wait_ge(hs[h], 1)         # px WAR vs ACT h0 read
                    te.wait_ge(gs[h][(t - 1) % 2], 16 * ((t - 1) // 2 + 1))
                    gb = gather[(t - 1) % 4]
                    for c, (pb, col) in enumerate((
                            (px[h], 0), (py[h], 0),
                            (px[h], BH), (py[h], BH))):
                        if c == 0 and t >= 3:
                            te.wait_ge(ak[h], AK(t - 1, 0))  # f WAR
                        if c == 1 and t >= 3:
                            te.wait_ge(ak[h], AK(t - 1, 1))  # i WAR
                        if c == 2:
                            te.wait_ge(ak[h], AK(t, 0))      # bankX: sig_f done
                        if c == 3:
                            te.wait_ge(ak[h], AK(t, 1))      # bankY: sig_i done
                        base = c * 8   # prep stores tiles in compute order
                        for k in range(NC):
                            last = k == NC - 1
                            ins = mm(pb[:, col:col + BH],
                                     W[:, (base + k) * 128:(base + k + 1) * 128],
                                     gb[:, k * B + BH * h:k * B + BH * h + BH],
                                     start=k == 0, stop=last)
                            if last:
                                ins.then_inc(gk[h])          # GK(t, c)
                if t >= 2:
                    outproj(t - 1)
            outproj(S_)

        @block.scalar
        def _(act):
            act.wait_ge(in_dma, 128)
            for h in range(2):
                act.wait_ge(gk[h], 1)
                act.activation(hT[h][0][:, :], px[h][:, 0:BH], AF.Identity,
                               bias=fcb_sb[:, 0:1]).then_inc(hs[h])
            for t in range(1, S_ + 1):
                for h in range(2):
                    act.wait_ge(gk[h], GK(t, 0))
                    if t >= 3:
                        act.wait_ge(dk[h], 4 * (t - 1) - 5)  # sfT WAR vs c2
                    act.activation(sfT[h][:, :], px[h][:, 0:BH], AF.Sigmoid,
                                   bias=bias_sb[:, 0:1]).then_inc(ak[h])
                    act.wait_ge(gk[h], GK(t, 1))
                    if t >= 3:
                        act.wait_ge(dk[h], 4 * (t - 1) - 4)  # siT WAR vs t1
                    act.activation(siT[h][:, :], py[h][:, 0:BH], AF.Sigmoid,
                                   bias=bias_sb[:, 1:2]).then_inc(ak[h])
                    act.wait_ge(gk[h], GK(t, 2))
                    act.activation(sgT[h][:, :], px[h][:, BH:2 * BH], AF.Tanh,
                                   bias=bias_sb[:, 2:3]).then_inc(ak[h])
                    act.wait_ge(gk[h], GK(t, 3))
                    if t >= 2:
                        act.wait_ge(dk[h], DVE_H(t - 1))     # soT WAR vs hT
                    act.activation(soT[h][:, :], py[h][:, BH:2 * BH],
                                   AF.Sigmoid,
                                   bias=bias_sb[:, 3:4]).then_inc(ak[h])
                    act.wait_ge(dk[h], DVE_C(t))
                    act.activation(tanhc[h][:, :], cT[h][:, :],
                                   AF.Tanh).then_inc(ak[h])

        @block.vector
        def _(dve):
            tt = dve.tensor_tensor
            dve.wait_ge(in_dma, 128)
            for t in range(1, S_ + 1):
                for h in range(2):
                    if t == 1:
                        dve.wait_ge(ak[h], AK(1, 2))
                        tt(cT[h][:, :], siT[h][:, :], sgT[h][:, :],
                           ALU.mult).then_inc(dk[h])          # c_1 = i*g
                    else:
                        dve.wait_ge(ak[h], AK(t, 0))
                        tt(c2[h][:, :], cT[h][:, :], sfT[h][:, :],
                           ALU.mult).then_inc(dk[h])          # 4t-5
                        dve.wait_ge(ak[h], AK(t, 2))
                        tt(t1[h][:, :], siT[h][:, :], sgT[h][:, :],
                           ALU.mult).then_inc(dk[h])          # 4t-4
                        tt(cT[h][:, :], c2[h][:, :], t1[h][:, :],
                           ALU.add).then_inc(dk[h])           # 4t-3
                    dve.wait_ge(ak[h], AK(t, 4))   # tanh_c (implies sig_o)
                    if t >= 2:
                        dve.wait_ge(ls[h], 16 * (t - 1))  # hT parity WAR
                    tt(hT[h][t % 2][:, :], soT[h][:, :], tanhc[h][:, :],
                       ALU.mult).then_inc(dk[h])              # DVE_H(t)
                if t >= 2:
                    j = t - 1
                    dve.wait_ge(osem, j)
                    dve.tensor_copy(out_acc[:, (j - 1) * OL:j * OL],
                                    pout[j % 2][:, :]).then_inc(ocp)  # = j
            dve.wait_ge(osem, S_)
            dve.tensor_copy(out_acc[:, (S_ - 1) * OL:S_ * OL],
                            pout[S_ % 2][:, :]).then_inc(ocp)  # = S

    ctx.close()
    nc.finalize()
    return nc


def _prep_inputs(latent, fc_w, fc_b, w_ih, w_hh, b_ih, b_hh, out_w, out_b):
    latent = np.asarray(latent, np.float32)
    fc_w = np.asarray(fc_w, np.float32)
    fc_b = np.asarray(fc_b, np.float32)
    w_ih = np.asarray(w_ih, np.float32)
    w_hh = np.asarray(w_hh, np.float32)
    b_ih = np.asarray(b_ih, np.float32)
    b_hh = np.asarray(b_hh, np.float32)
    out_w = np.asarray(out_w, np.float32)
    out_b = np.asarray(out_b, np.float32)

    wc = w_ih + w_hh
    biasc = b_ih + b_hh

    latT = np.zeros((128, 2 * B), np.float32)
    for tki in range(2):
        latT[:, tki * B:(tki + 1) * B] = latent[:, tki * 128:(tki + 1) * 128].T

    in_maps = []
    for j in range(NC):
        hsl = slice(HL * j, HL * (j + 1))
        wcT = np.zeros((128, 32 * 128), np.float32)
        whhT = np.zeros((128, 32 * 128), np.float32)
        biasT = np.zeros((128, 4), np.float32)
        for c, q in enumerate(CHUNK_Q):     # f, i, g, o
            rows = np.arange(q * H + HL * j, q * H + HL * (j + 1))
            biasT[:, c] = biasc[rows]
            for k in range(NC):
                ksl = slice(128 * k, 128 * (k + 1))
                wcT[:, (c * 8 + k) * 128:(c * 8 + k + 1) * 128] = \
                    wc[rows][:, ksl].T
                whhT[:, (c * 8 + k) * 128:(c * 8 + k + 1) * 128] = \
                    w_hh[rows][:, ksl].T
        outwT = np.zeros((128, NC * OL), np.float32)
        for k in range(NC):
            ksl = slice(128 * k, 128 * (k + 1))
            outwT[:, k * OL:(k + 1) * OL] = out_w[OL * j:OL * (j + 1), ksl].T
        fcwT = np.zeros((128, 2 * HL), np.float32)
        for tki in range(2):
            fcwT[:, tki * HL:(tki + 1) * HL] = \
                fc_w[hsl, tki * 128:(tki + 1) * 128].T
        misc = np.zeros((1, 2 * B), np.float32)
        misc[0, 0:B] = 1.0
        misc[0, B:B + OL] = out_b[OL * j:OL * (j + 1)]
        in_maps.append({
            "latT": latT.astype(BF16),
            "fcwT": fcwT.astype(BF16),
            "fcb": fc_b[hsl].reshape(128, 1).astype(np.float32),
            "wcT": wcT.astype(BF16),
            "whhT": whhT.astype(BF16),
            "outwT": outwT.astype(BF16),
            "biasT": biasT,
            "misc": misc.astype(BF16),
        })
    return in_maps


def _install_profile_shim():
    import types
    if 'antenv.axon_hooks' in sys.modules:
        return
    m = types.ModuleType('antenv.axon_hooks')
    m._hook = None
    m.set_axon_ntff_profile_hook = lambda h: setattr(m, '_hook', h)
    m.get_axon_ntff_profile_hook = lambda: m._hook
    sys.modules['antenv.axon_hooks'] = m
    try:
        import antenv
        antenv.axon_hooks = m
        from trn_agent_boot.trn_boot import _ntff_profile_via_ctypes
        m.set_axon_ntff_profile_hook(
            _ntff_profile_via_ctypes('/opt/axon/libaxon_pjrt.so'))
    except Exception:
        pass


_CACHE = {}


def kernel(latent, seq_len, fc_w, fc_b, w_ih, w_hh, b_ih, b_hh, out_w, out_b):
    # Recover cleanly if a previous run left the NeuronCores wedged.
    os.environ.setdefault("NEURON_RT_RESET_CORES", "1")
    from concourse import bass_utils

    s_len = int(seq_len)
    assert s_len == S, f"kernel hardcodes seq_len={S}, got {s_len}"

    if os.environ.get("BASS_TRACE"):
        _install_profile_shim()

    if "nc" not in _CACHE:
        _CACHE["nc"] = _build_nc(s_len)
    nc = _CACHE["nc"]

    in_maps = _prep_inputs(latent, fc_w, fc_b, w_ih, w_hh, b_ih, b_hh,
                           out_w, out_b)

    kw = {}
    if os.environ.get("BASS_TRACE"):
        import tempfile
        kw["trace"] = True
        kw["tmpdir"] = tempfile.mkdtemp(prefix="nn_decoder_")
        print(f"[kernel] trace tmpdir: {kw['tmpdir']}")
    res = bass_utils.run_bass_kernel_spmd(
        nc, in_maps, core_ids=list(range(NC)), **kw)
    if getattr(res, "exec_time_ns", None) is not None:
        print(f"[kernel] exec_time_ns: {res.exec_time_ns}")
        _CACHE["exec_time_ns"] = res.exec_time_ns

    outs = [np.asarray(res.results[j]["out"], np.float32).reshape(B, S, OL)
            for j in range(NC)]
    return np.concatenate(outs, axis=2)


# revision 3
# speedup vs baseline: 1.0232x; 1.0232x over previous
"""LSTM decoder (nn_Decoder) on 8 trn2 NeuronCores — dual-pipeline version.

Tensor-parallel over the gate dimension (each core owns a 128-row slice of
h/c) with the batch split into two independent half-recurrences (rows 0:32
and 32:64) run phase-shifted, so each half's h-broadcast flight hides under
the other half's compute.

Transposed layout throughout: partitions = this core's 128 h dims, free dim
= batch half. Gates are computed as gates.T chunks f,i,g,o, each [128, 32],
via 8 k-matmuls with the weight tile [128,128] bf16 as lhsT (stationary) and
the gathered hT slot [128, 32] as rhs — measured 25 ns/kmm. cT/hT stay
transposed, so there are no PE transposes, and gate biases ride the ACT
per-partition bias port (no bias matmuls).

PSUM banks: per half, bankX = f(cols 0:32)+g(32:64), bankY = i(0:32)+o(32:64)
so the PE never writes a bank an ACT read is pending on (g waits sig_f, o
waits sig_i). outproj double-buffers 2 more banks ([64,64], lhsT = gather
slot [128,64] spanning both halves, batch rows come out in order).

Exchange: per half per step one remote_dma_broadcast of hT [128,32] bf16
(8 KB) into slot-column pid*64 + 32*half of every core's gather ring (depth
4), on SWDGE queue <half>. Measured ring: trigger->all-arrived ~2.6 us.

Semaphores per half h: gs[h][parity] arrivals (+16/round), ls[h] local send
(+16), pp[h] prep, gk[h] gate chunks (h0=1; f,i,g,o of step t = 4t-2..4t+1),
ak[h] ACT ops (f,i,g,o,c of step t = 5t-4..5t), dk[h] DVE ops (t=1: c=1,
hT=2; t>=2: c2=4t-5, t1=4t-4, c=4t-3, hT=4t-2), hs[h] h0 ready.
Shared: osem/ocp outproj produce/copy (= j), in_dma, odma.
"""
import os
import sys

sys.path.insert(0, "/opt/trn_rl_repo")

import numpy as np
import ml_dtypes

BF16 = ml_dtypes.bfloat16

B = 64          # batch
BH = 32         # batch half
L = 256         # latent dim
H = 1024        # hidden
O = 512         # output dim
S = 256         # seq len
NC = 8          # cores
HL = H // NC    # 128, per-core h slice
OL = O // NC    # 64, per-core out slice

# gate chunk order (compute order): f first (c update), then i, g, o.
# chunk c -> quarter of the 4H gate dim in reference i,f,g,o order.
CHUNK_Q = [0, 2, 1, 3]   # i, g, f, o (t1-critical ops first)


def GK(t, c):
    return 4 * t - 2 + c          # chunk c in 0..3; h0 = 1


def AK(t, c):
    return 5 * t - 5 + c + 1      # c in 0..4 = f,i,g,o,tanh_c


def DVE_C(t):
    return 1 if t == 1 else 4 * t - 3


def DVE_H(t):
    return 2 if t == 1 else 4 * t - 2


def _build_nc(s_len):
    from concourse import bass, mybir
    from concourse import bacc
    from concourse.ap import AP
    from contextlib import ExitStack

    S_ = s_len
    nc = bacc.Bacc("TRN2", debug=False, num_swdge_queues=2)
    f32 = mybir.dt.float32
    bf16 = mybir.dt.bfloat16
    AF = mybir.ActivationFunctionType
    ALU = mybir.AluOpType

    d_lat = nc.dram_tensor("latT", [128, 2 * B], bf16, kind="ExternalInput")
    d_fcw = nc.dram_tensor("fcwT", [128, 2 * HL], bf16, kind="ExternalInput")
    d_fcb = nc.dram_tensor("fcb", [128, 1], f32, kind="ExternalInput")
    d_wc = nc.dram_tensor("wcT", [128, 32 * 128], bf16, kind="ExternalInput")
    d_whh = nc.dram_tensor("whhT", [128, 32 * 128], bf16, kind="ExternalInput")
    d_outw = nc.dram_tensor("outwT", [128, NC * OL], bf16, kind="ExternalInput")
    d_bias = nc.dram_tensor("biasT", [128, 4], f32, kind="ExternalInput")
    d_misc = nc.dram_tensor("misc", [1, 2 * B], bf16, kind="ExternalInput")
    d_out = nc.dram_tensor("out", [B, S_ * OL], f32, kind="ExternalOutput")

    ctx = ExitStack()
    sem = lambda n: ctx.enter_context(nc.semaphore(n))
    sb = lambda n, sh, dt: ctx.enter_context(nc.sbuf_tensor(n, sh, dt))
    ps = lambda n, sh, dt: ctx.enter_context(nc.psum_tensor(n, sh, dt))

    in_dma = sem("in_dma")     # 8 loads x 16 = 128
    gs = [[sem(f"gs{h}p{p}") for p in range(2)] for h in range(2)]
    ls = [sem(f"ls{h}") for h in range(2)]
    pp = [sem(f"pp{h}") for h in range(2)]
    gk = [sem(f"gk{h}") for h in range(2)]
    ak = [sem(f"ak{h}") for h in range(2)]
    dk = [sem(f"dk{h}") for h in range(2)]
    hs = [sem(f"hs{h}") for h in range(2)]
    osem = sem("osem")
    ocp = sem("ocp")
    odma = sem("odma")

    lat_sb = sb("lat_sb", [128, 2 * B], bf16)
    fcw_sb = sb("fcw_sb", [128, 2 * HL], bf16)
    fcb_sb = sb("fcb_sb", [128, 1], f32)
    wc_sb = sb("wc_sb", [128, 32 * 128], bf16)
    whh_sb = sb("whh_sb", [128, 32 * 128], bf16)
    outw_sb = sb("outw_sb", [128, NC * OL], bf16)
    bias_sb = sb("bias_sb", [128, 4], f32)
    misc_sb = sb("misc_sb", [1, 2 * B], bf16)
    gather = [sb(f"gather{r}", [128, NC * B], bf16) for r in range(4)]
    hT = [[sb(f"hT{h}p{p}", [128, BH], bf16) for p in range(2)]
          for h in range(2)]
    sfT = [sb(f"sfT{h}", [128, BH], f32) for h in range(2)]
    siT = [sb(f"siT{h}", [128, BH], f32) for h in range(2)]
    sgT = [sb(f"sgT{h}", [128, BH], f32) for h in range(2)]
    soT = [sb(f"soT{h}", [128, BH], f32) for h in range(2)]
    tanhc = [sb(f"tanhc{h}", [128, BH], f32) for h in range(2)]
    c2 = [sb(f"c2_{h}", [128, BH], f32) for h in range(2)]
    t1 = [sb(f"t1_{h}", [128, BH], f32) for h in range(2)]
    cT = [sb(f"cT{h}", [128, BH], f32) for h in range(2)]
    out_acc = sb("out_acc", [B, S_ * OL], f32)

    pA = [ps(f"pA{h}", [128, 2 * BH], f32) for h in range(2)]   # i | o
    pB = [ps(f"pB{h}", [128, BH], f32) for h in range(2)]       # f (+h0)
    pg = [ps(f"pg{h}", [128, BH], f32) for h in range(2)]       # g
    pout = [ps(f"pout{p}", [B, OL], f32) for p in range(2)]

    ones = lambda: misc_sb[0:1, 0:B]
    outb = lambda: misc_sb[0:1, B:B + OL]

    OUT_CHUNK = min(32, S_)
    n_chunks = (S_ + OUT_CHUNK - 1) // OUT_CHUNK

    with nc.Block() as block:

        @block.sync
        def _(sync):
            n = 0
            for dram, buf in ((d_lat, lat_sb), (d_fcw, fcw_sb),
                              (d_fcb, fcb_sb), (d_wc, wc_sb),
                              (d_whh, whh_sb), (d_outw, outw_sb),
                              (d_bias, bias_sb), (d_misc, misc_sb)):
                if n:
                    sync.wait_ge(in_dma, n)
                sync.dma_start(buf[:, :], dram[:, :]).then_inc(in_dma, 16)
                n += 16
            for ch in range(n_chunks):
                hi = min((ch + 1) * OUT_CHUNK, S_)   # steps 1..hi copied
                sync.wait_ge(ocp, hi)
                if ch:
                    sync.wait_ge(odma, 16 * ch)
                sync.dma_start(
                    d_out[:, ch * OUT_CHUNK * OL:hi * OL],
                    out_acc[:, ch * OUT_CHUNK * OL:hi * OL],
                ).then_inc(odma, 16)
            sync.wait_ge(odma, 16 * n_chunks)

        @block.gpsimd
        def _(gp):
            pid = gp.partition_id()
            dyn = [[AP(gather[r].ap().tensor, pid * B + BH * h,
                       gather[r][:, 0:BH].ap.copy()) for r in range(4)]
                   for h in range(2)]
            gp.wait_ge(in_dma, 128)
            gp.bir_kernel_barrier_wait([list(range(NC))])
            for i in range(S_ + 1):
                for h in range(2):
                    if i:
                        gp.wait_ge(ls[h], 16 * i)
                    gp.remote_dma_broadcast(
                        dyn[h][i % 4], hT[h][i % 2][:, :],
                        remote_sem=gs[h][i % 2], local_sem=ls[h],
                        rdests=[(0, k) for k in range(NC)],
                        queue_num=h,
                    ).then_inc(pp[h], 1)
                    gp.wait_ge(pp[h], i + 1)
                    if i == 0:
                        gp.wait_ge(hs[h], 1)
                    else:
                        gp.wait_ge(dk[h], DVE_H(i))
                    gp.trigger_dma(count=1, queue_num=h)

        @block.tensor
        def _(te):
            mm = te.matmul

            def outproj(j):
                po = pout[j % 2]
                # needs BOTH halves' exchange j arrived; half A's wait is
                # implied by program order, half B's is explicit (cheap —
                # B's round j lands well before A's round j+1 in steady
                # state, so this almost never stalls).
                te.wait_ge(gs[0][j % 2], 16 * (j // 2 + 1))
                te.wait_ge(gs[1][j % 2], 16 * (j // 2 + 1))
                if j > 2:
                    te.wait_ge(ocp, j - 2)
                mm(po[:, :], ones(), outb(), start=True, stop=False)
                gb = gather[j % 4]
                for k in range(NC):
                    last = k == NC - 1
                    ins = mm(po[:, :], gb[:, k * B:(k + 1) * B],
                             outw_sb[:, k * OL:(k + 1) * OL],
                             start=False, stop=last)
                    if last:
                        ins.then_inc(osem)           # osem = j

            te.wait_ge(in_dma, 128)
            for h in range(2):
                # h0T slice = fc_w @ latent.T + (fcb on ACT)
                mm(pB[h][:, 0:BH], fcw_sb[:, 0:HL],
                   lat_sb[:, BH * h:BH * h + BH], start=True, stop=False)
                mm(pB[h][:, 0:BH], fcw_sb[:, HL:2 * HL],
                   lat_sb[:, B + BH * h:B + BH * h + BH],
                   start=False, stop=True).then_inc(gk[h])      # gk = 1

            for t in range(1, S_ + 1):
                for h in range(2):
                    W = whh_sb if t == 1 else wc_sb
                    te.wait_ge(gs[h][(t - 1) % 2], 16 * ((t - 1) // 2 + 1))
                    gb = gather[(t - 1) % 4]
                    for c, (pb, col) in enumerate((
                            (pA[h], 0), (pg[h], 0),
                            (pB[h], 0), (pA[h], BH))):
                        if c == 0 and t >= 3:
                            te.wait_ge(ak[h], AK(t - 1, 0))  # i WAR
                        if c == 1 and t >= 3:
                            te.wait_ge(ak[h], AK(t - 1, 1))  # g WAR
                        if c == 2:
                            if t == 1:
                                te.wait_ge(hs[h], 1)  # pB WAR vs ACT h0 read
                            elif t >= 3:
                                te.wait_ge(ak[h], AK(t - 1, 2))  # f WAR
                        if c == 3:
                            te.wait_ge(ak[h], AK(t, 0))  # pA bank: sig_i done
                        base = c * 8   # prep stores tiles in compute order
                        for k in range(NC):
                            last = k == NC - 1
                            ins = mm(pb[:, col:col + BH],
                                     W[:, (base + k) * 128:(base + k + 1) * 128],
                                     gb[:, k * B + BH * h:k * B + BH * h + BH],
                                     start=k == 0, stop=last)
                            if last:
                                ins.then_inc(gk[h])          # GK(t, c)
                if t >= 2:
                    outproj(t - 1)
            outproj(S_)

        @block.scalar
        def _(act):
            act.wait_ge(in_dma, 128)
            for h in range(2):
                act.wait_ge(gk[h], 1)
                act.activation(hT[h][0][:, :], pB[h][:, 0:BH], AF.Identity,
                               bias=fcb_sb[:, 0:1]).then_inc(hs[h])
            for t in range(1, S_ + 1):
                for h in range(2):
                    act.wait_ge(gk[h], GK(t, 0))
                    if t >= 3:
                        act.wait_ge(dk[h], 4 * (t - 1) - 5)  # siT WAR vs t1
                    act.activation(siT[h][:, :], pA[h][:, 0:BH], AF.Sigmoid,
                                   bias=bias_sb[:, 0:1]).then_inc(ak[h])
                    act.wait_ge(gk[h], GK(t, 1))
                    if t >= 3:
                        act.wait_ge(dk[h], 4 * (t - 1) - 5)  # sgT WAR vs t1
                    act.activation(sgT[h][:, :], pg[h][:, 0:BH], AF.Tanh,
                                   bias=bias_sb[:, 1:2]).then_inc(ak[h])
                    act.wait_ge(gk[h], GK(t, 2))
                    if t >= 3:
                        act.wait_ge(dk[h], 4 * (t - 1) - 4)  # sfT WAR vs c2
                    act.activation(sfT[h][:, :], pB[h][:, 0:BH], AF.Sigmoid,
                                   bias=bias_sb[:, 2:3]).then_inc(ak[h])
                    act.wait_ge(gk[h], GK(t, 3))
                    if t >= 2:
                        act.wait_ge(dk[h], DVE_H(t - 1))     # soT WAR vs hT
                    act.activation(soT[h][:, :], pA[h][:, BH:2 * BH],
                                   AF.Sigmoid,
                                   bias=bias_sb[:, 3:4]).then_inc(ak[h])
                    act.wait_ge(dk[h], DVE_C(t))
                    act.activation(tanhc[h][:, :], cT[h][:, :],
                                   AF.Tanh).then_inc(ak[h])

        @block.vector
        def _(dve):
            tt = dve.tensor_tensor
            dve.wait_ge(in_dma, 128)
            for t in range(1, S_ + 1):
                for h in range(2):
                    if t == 1:
                        dve.wait_ge(ak[h], AK(1, 1))
                        tt(cT[h][:, :], siT[h][:, :], sgT[h][:, :],
                           ALU.mult).then_inc(dk[h])          # c_1 = i*g
                    else:
                        dve.wait_ge(ak[h], AK(t, 1))
                        tt(t1[h][:, :], siT[h][:, :], sgT[h][:, :],
                           ALU.mult).then_inc(dk[h])          # 4t-5
                        dve.wait_ge(ak[h], AK(t, 2))
                        tt(c2[h][:, :], cT[h][:, :], sfT[h][:, :],
                           ALU.mult).then_inc(dk[h])          # 4t-4
                        tt(cT[h][:, :], c2[h][:, :], t1[h][:, :],
                           ALU.add).then_inc(dk[h])           # 4t-3
                    dve.wait_ge(ak[h], AK(t, 4))   # tanh_c (implies sig_o)
                    if t >= 2:
                        dve.wait_ge(ls[h], 16 * (t - 1))  # hT parity WAR
                    tt(hT[h][t % 2][:, :], soT[h][:, :], tanhc[h][:, :],
                       ALU.mult).then_inc(dk[h])              # DVE_H(t)
                if t >= 2:
                    j = t - 1
                    dve.wait_ge(osem, j)
                    dve.tensor_copy(out_acc[:, (j - 1) * OL:j * OL],
                                    pout[j % 2][:, :]).then_inc(ocp)  # = j
            dve.wait_ge(osem, S_)
            dve.tensor_copy(out_acc[:, (S_ - 1) * OL:S_ * OL],
                            pout[S_ % 2][:, :]).then_inc(ocp)  # = S

    ctx.close()
    nc.finalize()
    return nc


def _prep_inputs(latent, fc_w, fc_b, w_ih, w_hh, b_ih, b_hh, out_w, out_b):
    latent = np.asarray(latent, np.float32)
    fc_w = np.asarray(fc_w, np.float32)
    fc_b = np.asarray(fc_b, np.float32)
    w_ih = np.asarray(w_ih, np.float32)
    w_hh = np.asarray(w_hh, np.float32)
    b_ih = np.asarray(b_ih, np.float32)
    b_hh = np.asarray(b_hh, np.float32)
    out_w = np.asarray(out_w, np.float32)
    out_b = np.asarray(out_b, np.float32)

    wc = w_ih + w_hh
    biasc = b_ih + b_hh

    latT = np.zeros((128, 2 * B), np.float32)
    for tki in range(2):
        latT[:, tki * B:(tki + 1) * B] = latent[:, tki * 128:(tki + 1) * 128].T

    in_maps = []
    for j in range(NC):
        hsl = slice(HL * j, HL * (j + 1))
        wcT = np.zeros((128, 32 * 128), np.float32)
        whhT = np.zeros((128, 32 * 128), np.float32)
        biasT = np.zeros((128, 4), np.float32)
        for c, q in enumerate(CHUNK_Q):     # f, i, g, o
            rows = np.arange(q * H + HL * j, q * H + HL * (j + 1))
            biasT[:, c] = biasc[rows]
            for k in range(NC):
                ksl = slice(128 * k, 128 * (k + 1))
                wcT[:, (c * 8 + k) * 128:(c * 8 + k + 1) * 128] = \
                    wc[rows][:, ksl].T
                whhT[:, (c * 8 + k) * 128:(c * 8 + k + 1) * 128] = \
                    w_hh[rows][:, ksl].T
        outwT = np.zeros((128, NC * OL), np.float32)
        for k in range(NC):
            ksl = slice(128 * k, 128 * (k + 1))
            outwT[:, k * OL:(k + 1) * OL] = out_w[OL * j:OL * (j + 1), ksl].T
        fcwT = np.zeros((128, 2 * HL), np.float32)
        for tki in range(2):
            fcwT[:, tki * HL:(tki + 1) * HL] = \
                fc_w[hsl, tki * 128:(tki + 1) * 128].T
        misc = np.zeros((1, 2 * B), np.float32)
        misc[0, 0:B] = 1.0
        misc[0, B:B + OL] = out_b[OL * j:OL * (j + 1)]
        in_maps.append({
            "latT": latT.astype(BF16),
            "fcwT": fcwT.astype(BF16),
            "fcb": fc_b[hsl].reshape(128, 1).astype(np.float32),
            "wcT": wcT.astype(BF16),
            "whhT": whhT.astype(BF16),
            "outwT": outwT.astype(BF16),
            "biasT": biasT,
            "misc": misc.astype(BF16),
        })
    return in_maps


def _install_profile_shim():
    import types
    if 'antenv.axon_hooks' in sys.modules:
        return
    m = types.ModuleType('antenv.axon_hooks')
    m._hook = None
    m.set_axon_ntff_profile_hook = lambda h: setattr(m, '_hook', h)
    m.get_axon_ntff_profile_hook = lambda: m._hook
    sys.modules['antenv.axon_hooks'] = m
    try:
        import antenv
        antenv.axon_hooks = m
        from trn_agent_boot.trn_boot import _ntff_profile_via_ctypes
        m.set_axon_ntff_profile_hook(
            _ntff_profile_via_ctypes('/opt/axon/libaxon_pjrt.so'))
    except Exception:
        pass


_CACHE = {}


def kernel(latent, seq_len, fc_w, fc_b, w_ih, w_hh, b_ih, b_hh, out_w, out_b):
    # Recover cleanly if a previous run left the NeuronCores wedged.
    os.environ.setdefault("NEURON_RT_RESET_CORES", "1")
    from concourse import bass_utils

    s_len = int(seq_len)
    assert s_len == S, f"kernel hardcodes seq_len={S}, got {s_len}"

    if os.environ.get("BASS_TRACE"):
        _install_profile_shim()

    if "nc" not in _CACHE:
        _CACHE["nc"] = _build_nc(s_len)
    nc = _CACHE["nc"]

    in_maps = _prep_inputs(latent, fc_w, fc_b, w_ih, w_hh, b_ih, b_hh,
                           out_w, out_b)

    kw = {}
    if os.environ.get("BASS_TRACE"):
        import tempfile
        kw["trace"] = True
        kw["tmpdir"] = tempfile.mkdtemp(prefix="nn_decoder_")
        print(f"[kernel] trace tmpdir: {kw['tmpdir']}")
    res = bass_utils.run_bass_kernel_spmd(
        nc, in_maps, core_ids=list(range(NC)), **kw)
    if getattr(res, "exec_time_ns", None) is not None:
        print(f"[kernel] exec_time_ns: {res.exec_time_ns}")
        _CACHE["exec_time_ns"] = res.exec_time_ns

    outs = [np.asarray(res.results[j]["out"], np.float32).reshape(B, S, OL)
            for j in range(NC)]
    return np.concatenate(outs, axis=2)
